# revision 20
# baseline (speedup 1.0000x reference)
"""DSTAGNN block kernel for 8x TRN2 NeuronCores (batch-parallel SPMD).

Per core: 2 batches. Stages:
  A: temporal embedding + attention (tiny (12,1024) tiles)
  B: pre_conv + spatial embedding + SQ/SK projections
  C: spatial attention scores + softmax-over-n + cheb graph contraction,
     streamed over (mhalf, k) chunks of the (N,N) domain; softmax denominator
     via matmul with expS as stationary weights; normalization deferred to
     after the contraction (divide the 12-col result by the ones-column).
  D: theta contraction -> GTU convs (dt pairs packed on 128 partitions via a
     t-shifted X copy) -> gates -> fcw via host-packed block-diag weights on a
     DMA-reshuffled (96,16k) layout -> PE transpose back to n-partitions ->
     residual + relu + LayerNorm over C -> output.
Host prep inside kernel(): weight repacking only (transposes/tiling/padding,
adj_pa*cmask fusion). All batch-dependent math runs on device.
"""

import numpy as np
import ml_dtypes

import concourse.bacc as bacc
import concourse.bass as bass
import concourse.mybir as mybir
import concourse.tile as tile
from concourse.bass_utils import run_bass_kernel_spmd

F32 = mybir.dt.float32
BF16 = mybir.dt.bfloat16
AF = mybir.ActivationFunctionType
OP = mybir.AluOpType
AX = mybir.AxisListType

B, N, T, H, DK, DM, K, C = 16, 1024, 12, 3, 32, 512, 3, 64
NCORES = 8
BPC = B // NCORES
NCH = N // 128
MH = N // 512
TAU = 3 * T - 12  # 24
RSQ = 1.0 / float(np.sqrt(DK))
EPS = 1e-5

_PROG = None


def _ln_free(nc, sb, x_ap, P, D, g_bc=None, b_bc=None):
    """In-place LayerNorm over the flat free axis of x_ap (P, D)."""
    nsum = sb.tile([P, 1], F32, tag="ln_s")
    nc.vector.tensor_reduce(nsum[:], x_ap, axis=AX.X, op=OP.add, negate=True)
    nmean = sb.tile([P, 1], F32, tag="ln_m")
    nc.vector.tensor_scalar_mul(nmean[:], nsum[:], 1.0 / D)
    nc.vector.tensor_scalar_add(x_ap, x_ap, nmean[:])
    sq = sb.tile([P, D], F32, tag="ln_q")
    nc.scalar.square(sq[:], x_ap)
    var = sb.tile([P, 1], F32, tag="ln_v")
    nc.vector.tensor_reduce(var[:], sq[:], axis=AX.X, op=OP.add)
    ve = sb.tile([P, 1], F32, tag="ln_e")
    nc.vector.tensor_scalar(ve[:], var[:], 1.0 / D, EPS, op0=OP.mult, op1=OP.add)
    sd = sb.tile([P, 1], F32, tag="ln_d")
    nc.scalar.sqrt(sd[:], ve[:])
    rstd = sb.tile([P, 1], F32, tag="ln_r")
    nc.vector.reciprocal(rstd[:], sd[:])
    nc.vector.tensor_scalar_mul(x_ap, x_ap, rstd[:])
    if g_bc is not None:
        nc.vector.tensor_mul(x_ap, x_ap, g_bc)
        nc.vector.tensor_add(x_ap, x_ap, b_bc)


def build_program():
    nc = bacc.Bacc("TRN2", target_bir_lowering=False, debug=False)

    def din(name, shape, dt=F32):
        return nc.dram_tensor(name, shape, dt, kind="ExternalInput")

    xb_d = din("xb", [BPC, N, T])
    ra_d = din("ra", [BPC, H, T, T])
    raT_d = din("raT", [BPC, H, T, T])
    posT_d = din("posT", [T, N])
    gT_d = din("gTr", [1, N]); bT_d = din("bTr", [1, N])
    wq_d = din("wq", [N, H * DK]); wk_d = din("wk", [N, H * DK]); wv_d = din("wv", [N, H * DK])
    fc_d = din("fc", [H * DK, N])
    prewT_d = din("prewT", [T, DM])
    posSb_d = din("posSb", [N, DM])
    gS_d = din("gSr", [1, DM]); bS_d = din("bSr", [1, DM])
    swq_d = din("swq", [DM, H * DK]); swk_d = din("swk", [DM, H * DK])
    Mh_d = din("Mh", [K, N, N])
    cheb_d = din("cheb", [K, N, N])
    TH_d = din("TH", [96, T, 128])
    wp_d = {kw: din(f"wp{kw}", [128, (kw + 1) // 2, 128], BF16) for kw in (3, 5, 7)}
    gb_d = din("gbias", [128, 3])  # g3b/g5b/g7b columns
    fcw4_d = din("fcw4", [96, 48], BF16)
    rw_d = din("rwRow", [1, C * T]); rb_d = din("rbRow", [1, C * T])
    gln_d = din("glnRow", [1, C * T]); bln_d = din("blnRow", [1, C * T])
    id_d = din("idm", [128, 128])

    out_d = nc.dram_tensor("out", [BPC, N, C, T], F32, kind="ExternalOutput")
    sco_d = nc.dram_tensor("sco", [BPC, H, T, T], F32, kind="ExternalOutput")

    with tile.TileContext(nc) as tc:
        with (
            tc.tile_pool(name="const", bufs=1) as cp,
            tc.tile_pool(name="persist", bufs=1) as pp,
            tc.tile_pool(name="small", bufs=2) as sm,
            tc.tile_pool(name="ps", bufs=3, space=bass.MemorySpace.PSUM) as ps,
            tc.tile_pool(name="psacc", bufs=2, space=bass.MemorySpace.PSUM) as psa,
        ):
            idm = cp.tile([128, 128], F32)
            nc.sync.dma_start(idm[:], id_d[:])

            idm_bf = cp.tile([128, 128], BF16, name="idm_bf")
            nc.vector.tensor_copy(idm_bf[:], idm[:])

            def tr(out_ap, in_ap):
                p = in_ap.shape[0]
                ident = idm_bf if in_ap.dtype == BF16 else idm
                nc.tensor.transpose(out_ap, in_ap, ident[0:p, 0:p])
            TH_s = cp.tile([96, T, 128], F32, name="TH_s")
            nc.sync.dma_start(TH_s[:], TH_d[:])
            wp = {}
            for kw in (3, 5, 7):
                g = (kw + 1) // 2
                wp[kw] = cp.tile([128, g, 128], BF16, name=f"wp{kw}")
                nc.sync.dma_start(wp[kw][:], wp_d[kw][:])
            gb = cp.tile([128, 3], F32)
            nc.sync.dma_start(gb[:], gb_d[:])
            fcw4 = cp.tile([96, 48], BF16)
            nc.sync.dma_start(fcw4[:], fcw4_d[:])

            def bcast_row(dram, P, D, pool):
                row = pool.tile([1, D], F32, tag=dram.name + "_r")
                nc.sync.dma_start(row[:], dram[:])
                full = pool.tile([P, D], F32, tag=dram.name + "_f")
                nc.gpsimd.partition_broadcast(full[:], row[:])
                return full

            rw_bc = bcast_row(rw_d, 128, C * T, cp)
            rb_bc = bcast_row(rb_d, 128, C * T, cp)
            gln_bc = bcast_row(gln_d, 128, C * T, cp)
            bln_bc = bcast_row(bln_d, 128, C * T, cp)

            xnat = [pp.tile([128, NCH, T], F32, tag=f"xnat{b}", name=f"xnat{b}") for b in range(BPC)]
            xaug = [pp.tile([128, NCH, T + 1], F32, tag=f"xaug{b}", name=f"xaug{b}") for b in range(BPC)]
            SQt = [[pp.tile([DK, N], F32, tag=f"sqt{b}_{k}", name=f"sqt{b}_{k}") for k in range(K)] for b in range(BPC)]
            SKt = [[pp.tile([DK, N], F32, tag=f"skt{b}_{k}", name=f"skt{b}_{k}") for k in range(K)] for b in range(BPC)]
            rn3 = [[pp.tile([128, 96], F32, tag=f"rn3_{b}_{ms}", name=f"rn3_{b}_{ms}") for ms in range(NCH)] for b in range(BPC)]
            for b in range(BPC):
                for ms in range(NCH):
                    nc.vector.memset(rn3[b][ms][:], 0.0)

            # ================= STAGE A + B =================
            with tc.tile_pool(name="ab", bufs=1) as ab, tc.tile_pool(name="abw", bufs=2) as abw:
                posT = ab.tile([T, N], F32)
                nc.sync.dma_start(posT[:], posT_d[:])
                prewT = ab.tile([T, DM], F32)
                nc.sync.dma_start(prewT[:], prewT_d[:])
                gT_bc = bcast_row(gT_d, T, N, ab)
                bT_bc = bcast_row(bT_d, T, N, ab)
                gS_bc = bcast_row(gS_d, 128, DM, ab)
                bS_bc = bcast_row(bS_d, 128, DM, ab)
                wq_s = ab.tile([128, NCH, H * DK], F32)
                wk_s = ab.tile([128, NCH, H * DK], F32)
                wv_s = ab.tile([128, NCH, H * DK], F32)
                for wdst, wsrc in ((wq_s, wq_d), (wk_s, wk_d), (wv_s, wv_d)):
                    nc.sync.dma_start(wdst[:], wsrc[:].rearrange("(a p) k -> p a k", p=128))
                fc_s = ab.tile([H * DK, N], F32)
                nc.sync.dma_start(fc_s[:], fc_d[:])
                swq_s = ab.tile([128, 4, H * DK], F32)
                swk_s = ab.tile([128, 4, H * DK], F32)
                nc.sync.dma_start(swq_s[:], swq_d[:].rearrange("(a p) k -> p a k", p=128))
                nc.sync.dma_start(swk_s[:], swk_d[:].rearrange("(a p) k -> p a k", p=128))
                posSb_s = ab.tile([128, NCH, DM], F32)
                nc.sync.dma_start(posSb_s[:], posSb_d[:].rearrange("(a p) d -> p a d", p=128))

                for b in range(BPC):
                    nc.sync.dma_start(xnat[b][:], xb_d[b].rearrange("(a p) t -> p a t", p=128))
                    nc.vector.tensor_copy(xaug[b][:, :, 0:T], xnat[b][:])
                    nc.vector.memset(xaug[b][:, :, T:T + 1], 1.0)

                    TEmx = abw.tile([T, N], F32, tag="TEmx", bufs=1)
                    for a in range(NCH):
                        tp = ps.tile([T, 128], F32, tag="ps")
                        tr(tp[:], xnat[b][:, a, :])
                        nc.scalar.copy(TEmx[:, a * 128:(a + 1) * 128], tp[:])
                    nc.vector.tensor_add(TEmx[:], TEmx[:], posT[:])
                    _ln_free(nc, abw, TEmx[:], T, N, gT_bc[:], bT_bc[:])
                    TEmxT = abw.tile([128, NCH, T], F32, tag="TEmxT")
                    for a in range(NCH):
                        tp = ps.tile([128, T], F32, tag="ps")
                        tr(tp[:], TEmx[:, a * 128:(a + 1) * 128])
                        nc.scalar.copy(TEmxT[:, a, :], tp[:])
                    qkv = []
                    for w_s in (wq_s, wk_s, wv_s):
                        acc = psa.tile([T, H * DK], F32, tag="acc")
                        for a in range(NCH):
                            nc.tensor.matmul(acc[:], TEmxT[:, a, :], w_s[:, a, :],
                                             start=(a == 0), stop=(a == NCH - 1))
                        sb_t = abw.tile([T, H * DK], F32, tag=f"qkv{len(qkv)}")
                        nc.scalar.copy(sb_t[:], acc[:])
                        qkv.append(sb_t)
                    Q, Kh, V = qkv
                    QT = abw.tile([DK, H, T], F32, tag="QT")
                    KT = abw.tile([DK, H, T], F32, tag="KT")
                    for h in range(H):
                        tp = ps.tile([DK, T], F32, tag="ps")
                        tr(tp[:], Q[:, h * DK:(h + 1) * DK])
                        nc.scalar.mul(QT[:, h, :], tp[:], RSQ)
                        tp2 = ps.tile([DK, T], F32, tag="ps")
                        tr(tp2[:], Kh[:, h * DK:(h + 1) * DK])
                        nc.scalar.copy(KT[:, h, :], tp2[:])
                    ctx = abw.tile([T, H * DK], F32, tag="ctx")
                    ra_n = abw.tile([T, H, T], F32, tag="ra_n")
                    raT_n = abw.tile([T, H, T], F32, tag="raT_n")
                    for h in range(H):
                        nc.sync.dma_start(ra_n[:, h, :], ra_d[b, h])
                        nc.sync.dma_start(raT_n[:, h, :], raT_d[b, h])
                    for h in range(H):
                        sps = ps.tile([T, T], F32, tag="ps")
                        nc.tensor.matmul(sps[:], QT[:, h, :], KT[:, h, :], start=True, stop=True)
                        s_out = abw.tile([T, T], F32, tag="s_out")
                        nc.vector.tensor_add(s_out[:], sps[:], ra_n[:, h, :])
                        nc.sync.dma_start(sco_d[b, h], s_out[:])
                        tps = ps.tile([T, T], F32, tag="ps")
                        nc.tensor.matmul(tps[:], KT[:, h, :], QT[:, h, :], start=True, stop=True)
                        sT = abw.tile([T, T], F32, tag="sT")
                        nc.vector.tensor_add(sT[:], tps[:], raT_n[:, h, :])
                        mx = abw.tile([T, 1], F32, tag="smx")
                        nc.vector.tensor_reduce(mx[:], sT[:], axis=AX.X, op=OP.max, negate=True)
                        nc.scalar.activation(sT[:], sT[:], AF.Exp, bias=mx[:])
                        ssum = abw.tile([T, 1], F32, tag="ssum")
                        nc.vector.tensor_reduce(ssum[:], sT[:], axis=AX.X, op=OP.add)
                        rs = abw.tile([T, 1], F32, tag="rs")
                        nc.vector.reciprocal(rs[:], ssum[:])
                        nc.vector.tensor_scalar_mul(sT[:], sT[:], rs[:])
                        cps = ps.tile([T, DK], F32, tag="ps")
                        nc.tensor.matmul(cps[:], sT[:], V[:, h * DK:(h + 1) * DK],
                                         start=True, stop=True)
                        nc.scalar.copy(ctx[:, h * DK:(h + 1) * DK], cps[:])
                    ctxT = abw.tile([H * DK, T], F32, tag="ctxT")
                    tp = ps.tile([H * DK, T], F32, tag="ps")
                    tr(tp[:], ctx[:])
                    nc.scalar.copy(ctxT[:], tp[:])
                    TAT = abw.tile([T, N], F32, tag="TAT", bufs=1)
                    for mh in range(MH):
                        aps = ps.tile([T, 512], F32, tag="ps")
                        nc.tensor.matmul(aps[:], ctxT[:], fc_s[:, mh * 512:(mh + 1) * 512],
                                         start=True, stop=True)
                        nc.vector.tensor_add(TAT[:, mh * 512:(mh + 1) * 512], aps[:],
                                             TEmx[:, mh * 512:(mh + 1) * 512])
                    _ln_free(nc, abw, TAT[:], T, N)
                    SEmx = abw.tile([128, NCH, DM], F32, tag="SEmx", bufs=1)
                    for a in range(NCH):
                        aps = psa.tile([128, DM], F32, tag="acc")
                        nc.tensor.matmul(aps[:], TAT[:, a * 128:(a + 1) * 128], prewT[:],
                                         start=True, stop=True)
                        nc.vector.tensor_add(SEmx[:, a, :], aps[:], posSb_s[:, a, :])
                        _ln_free(nc, abw, SEmx[:, a, :], 128, DM, gS_bc[:], bS_bc[:])
                    SEmxT = abw.tile([128, 4, N], F32, tag="SEmxT", bufs=1)
                    for a in range(NCH):
                        for d in range(4):
                            tp2 = ps.tile([128, 128], F32, tag="ps")
                            tr(tp2[:], SEmx[:, a, d * 128:(d + 1) * 128])
                            nc.scalar.copy(SEmxT[:, d, a * 128:(a + 1) * 128], tp2[:])
                    for w_s, dst, scale in ((swq_s, SQt[b], RSQ), (swk_s, SKt[b], 1.0)):
                        for mh in range(MH):
                            acc = psa.tile([H * DK, 512], F32, tag="acc")
                            for d in range(4):
                                nc.tensor.matmul(acc[:], w_s[:, d, :],
                                                 SEmxT[:, d, mh * 512:(mh + 1) * 512],
                                                 start=(d == 0), stop=(d == 3))
                            for k in range(K):
                                if scale != 1.0:
                                    nc.scalar.mul(dst[k][:, mh * 512:(mh + 1) * 512],
                                                  acc[k * DK:(k + 1) * DK, :], scale)
                                else:
                                    nc.scalar.copy(dst[k][:, mh * 512:(mh + 1) * 512],
                                                   acc[k * DK:(k + 1) * DK, :])

            # ================= STAGE C =================
            with tc.tile_pool(name="cio", bufs=2) as cio, tc.tile_pool(name="cw", bufs=3) as cw:
                for mh in range(MH):
                    for k in range(K):
                        Mt = cio.tile([128, NCH, 512], F32, tag="Mt")
                        nc.sync.dma_start(Mt[:], Mh_d[k, :, mh * 512:(mh + 1) * 512]
                                          .rearrange("(a p) m -> p a m", p=128))
                        Ct = cio.tile([128, NCH, 512], F32, tag="Ct")
                        nc.sync.dma_start(Ct[:], cheb_d[k, :, mh * 512:(mh + 1) * 512]
                                          .rearrange("(a p) m -> p a m", p=128))
                        for b in range(BPC):
                            u_all = cw.tile([128, NCH, 512], F32, tag="u_all", bufs=1)
                            es_all = cw.tile([128, NCH, 512], F32, tag="es_all", bufs=1)
                            for a in range(NCH):
                                sps = ps.tile([128, 512], F32, tag="ps")
                                nc.tensor.matmul(sps[:], SQt[b][k][:, a * 128:(a + 1) * 128],
                                                 SKt[b][k][:, mh * 512:(mh + 1) * 512],
                                                 start=True, stop=True)
                                s2 = cw.tile([128, 512], F32, tag="s2")
                                nc.vector.tensor_add(s2[:], sps[:], Mt[:, a, :])
                                nc.scalar.activation(es_all[:, a, :], s2[:], AF.Exp)
                                nc.gpsimd.tensor_mul(u_all[:, a, :], es_all[:, a, :],
                                                     Ct[:, a, :])
                            racc = psa.tile([128, 4, 14], F32, tag="racc")
                            for ms in range(4):
                                sl = slice(ms * 128, (ms + 1) * 128)
                                for a in range(NCH):
                                    nc.tensor.matmul(racc[:, ms, 0:13], u_all[:, a, sl],
                                                     xaug[b][:, a, :],
                                                     start=(a == 0), stop=(a == NCH - 1))
                                for a in range(NCH):
                                    nc.tensor.matmul(racc[:, ms, 13:14], es_all[:, a, sl],
                                                     xaug[b][:, a, T:T + 1],
                                                     start=(a == 0), stop=(a == NCH - 1))
                            for ms in range(4):
                                inv = cw.tile([128, 1], F32, tag="inv")
                                nc.vector.reciprocal(inv[:], racc[:, ms, 13:14])
                                nc.vector.tensor_scalar(
                                    rn3[b][mh * 4 + ms][:, 32 * k:32 * k + 12],
                                    racc[:, ms, 0:12], inv[:], None, op0=OP.mult)

            # ================= STAGE D =================
            with tc.tile_pool(name="dbig", bufs=1) as db, tc.tile_pool(name="dw", bufs=3) as dw:
                tau0 = {3: 0, 5: T - 2, 7: 2 * T - 6}
                for b in range(BPC):
                    rT = db.tile([96, N], F32, tag="rT")
                    for ms in range(NCH):
                        tp = ps.tile([96, 128], F32, tag="ps")
                        tr(tp[:], rn3[b][ms][:])
                        nc.vector.tensor_copy(rT[:, ms * 128:(ms + 1) * 128], tp[:])
                    Xs = db.tile([128, T + 1, N], BF16, tag="dA", name="Xs")
                    nc.vector.memset(Xs[64:128, T - 1, :], 0.0)
                    nc.vector.memset(Xs[:, T, :], 0.0)
                    for t in range(T):
                        for mh in range(MH):
                            xps = ps.tile([128, 512], F32, tag="ps")
                            nc.tensor.matmul(xps[:], TH_s[:, t, :], rT[:, mh * 512:(mh + 1) * 512],
                                             start=True, stop=True)
                            sl = slice(mh * 512, (mh + 1) * 512)
                            nc.scalar.activation(Xs[0:64, t, sl], xps[0:64, :], AF.Relu)
                            if t > 0:
                                nc.scalar.activation(Xs[64:128, t - 1, sl], xps[64:128, :],
                                                     AF.Relu)
                    tc_t = db.tile([C, TAU, N], BF16, tag="dB", name="tc_t")
                    for ki, kw in enumerate((3, 5, 7)):
                        ng = (kw + 1) // 2
                        for tp_ in range(T - kw + 1):
                            for mh in range(MH):
                                yps = ps.tile([128, 512], F32, tag="ps")
                                for g in range(ng):
                                    nc.tensor.matmul(yps[:], wp[kw][:, g, :],
                                                     Xs[:, tp_ + 2 * g, mh * 512:(mh + 1) * 512],
                                                     start=(g == 0), stop=(g == ng - 1))
                                g1 = dw.tile([C, 512], BF16, tag="g1")
                                nc.scalar.activation(g1[:], yps[0:64, :], AF.Tanh,
                                                     bias=gb[0:64, ki:ki + 1])
                                g2 = dw.tile([C, 512], BF16, tag="g2")
                                nc.scalar.activation(g2[:], yps[64:128, :], AF.Sigmoid,
                                                     bias=gb[64:128, ki:ki + 1])
                                nc.vector.tensor_mul(
                                    tc_t[:, tau0[kw] + tp_, mh * 512:(mh + 1) * 512],
                                    g1[:], g2[:])
                    t4 = db.tile([96, 16, N], BF16, tag="dA", name="t4")
                    for j in range(4):
                        for tau in range(TAU):
                            nc.sync.dma_start(
                                t4[24 * j + tau:24 * j + tau + 1, :, :],
                                tc_t[16 * j:16 * (j + 1), tau, :])
                    tconv4 = db.tile([48, 16 * N], BF16, tag="dB", name="tconv4")
                    t4f = t4[:].rearrange("p a m -> p (a m)")
                    for ch in range(32):
                        fps = ps.tile([48, 512], F32, tag="ps")
                        nc.tensor.matmul(fps[:], fcw4[:], t4f[:, ch * 512:(ch + 1) * 512],
                                         start=True, stop=True)
                        nc.scalar.copy(tconv4[:, ch * 512:(ch + 1) * 512], fps[:])
                    tcv = tconv4[:].rearrange("p (c m) -> p c m", c=16)
                    for mc in range(NCH):
                        Ft = dw.tile([128, C * T], F32, tag="Ft")
                        Ftv = Ft[:].rearrange("p (j cc t) -> p j cc t", j=4, cc=16)
                        for cp_ in range(16):
                            tp2 = ps.tile([128, 48], BF16, tag="psbf", bufs=1)
                            tr(tp2[:], tcv[:, cp_, mc * 128:(mc + 1) * 128])
                            src = tp2[:].rearrange("p (j t) -> p j t", j=4)
                            if cp_ % 2 == 0:
                                nc.vector.tensor_copy(Ftv[:, :, cp_, :], src)
                            else:
                                nc.scalar.copy(Ftv[:, :, cp_, :], src)
                        xbc = xnat[b][:, mc, :].unsqueeze(1).broadcast_to([128, C, T])
                        xr = dw.tile([128, C * T], F32, tag="xr")
                        nc.vector.tensor_tensor(xr[:].rearrange("p (c t) -> p c t", c=C), xbc,
                                                rw_bc[:].rearrange("p (c t) -> p c t", c=C),
                                                op=OP.mult)
                        nc.vector.tensor_add(xr[:], xr[:], rb_bc[:])
                        nc.vector.tensor_add(Ft[:], Ft[:], xr[:])
                        nc.scalar.activation(Ft[:], Ft[:], AF.Relu)
                        # LN over c: strided view (p, t, c) with t-stride 1, c-stride T
                        ov = Ft[:].rearrange("p (c t) -> p t c", c=C)
                        nmean = dw.tile([128, T], F32, tag="nm")
                        nc.vector.tensor_reduce(nmean[:].unsqueeze(2), ov, axis=AX.X,
                                                op=OP.add, negate=True)
                        nc.vector.tensor_scalar_mul(nmean[:], nmean[:], 1.0 / C)
                        nc.vector.tensor_tensor(
                            Ft[:].rearrange("p (c t) -> p c t", c=C),
                            Ft[:].rearrange("p (c t) -> p c t", c=C),
                            nmean[:].unsqueeze(1).broadcast_to([128, C, T]), op=OP.add)
                        sq = dw.tile([128, C * T], F32, tag="sqF")
                        nc.scalar.square(sq[:], Ft[:])
                        var = dw.tile([128, T], F32, tag="vF")
                        nc.vector.tensor_reduce(var[:].unsqueeze(2),
                                                sq[:].rearrange("p (c t) -> p t c", c=C),
                                                axis=AX.X, op=OP.add)
                        ve = dw.tile([128, T], F32, tag="veF")
                        nc.vector.tensor_scalar(ve[:], var[:], 1.0 / C, EPS, op0=OP.mult, op1=OP.add)
                        sd = dw.tile([128, T], F32, tag="sdF")
                        nc.scalar.sqrt(sd[:], ve[:])
                        rstd = dw.tile([128, T], F32, tag="rsF")
                        nc.vector.reciprocal(rstd[:], sd[:])
                        nc.vector.tensor_tensor(
                            Ft[:].rearrange("p (c t) -> p c t", c=C),
                            Ft[:].rearrange("p (c t) -> p c t", c=C),
                            rstd[:].unsqueeze(1).broadcast_to([128, C, T]), op=OP.mult)
                        nc.vector.tensor_mul(Ft[:], Ft[:], gln_bc[:])
                        nc.vector.tensor_add(Ft[:], Ft[:], bln_bc[:])
                        nc.sync.dma_start(out_d[b, mc * 128:(mc + 1) * 128],
                                          Ft[:].rearrange("p (c t) -> p c t", c=C))

    nc.compile()
    return nc


def _host_prep(inputs):
    g = {k: np.asarray(v) for k, v in inputs.items()}
    x = np.ascontiguousarray(g["x"].reshape(B, N, T), np.float32)
    ra = np.ascontiguousarray(g["res_att"].reshape(B, H, T, T), np.float32)
    kwmap = {3: (g["g3w"], g["g3b"]), 5: (g["g5w"], g["g5b"]), 7: (g["g7w"], g["g7b"])}
    wp = {}
    for kw in (3, 5, 7):
        ngr = (kw + 1) // 2
        w = np.zeros((128, ngr, 128), np.float32)
        src = kwmap[kw][0].reshape(2 * C, C, kw)
        for gi in range(ngr):
            for hh in range(2):
                dt = 2 * gi + hh
                if dt < kw:
                    w[64 * hh:64 * hh + 64, gi, :] = src[:, :, dt].T
        wp[kw] = w.astype(ml_dtypes.bfloat16)
    gbias = np.stack([kwmap[3][1], kwmap[5][1], kwmap[7][1]], axis=1).astype(np.float32)
    fcw4 = np.zeros((96, 48), np.float32)
    for j in range(4):
        fcw4[24 * j:24 * j + 24, 12 * j:12 * j + 12] = g["fcw"]
    TH = np.zeros((96, T, 128), np.float32)
    for t in range(T):
        for k in range(K):
            TH[32 * k + t, t, 0:C] = g["theta"][k, 0, :]
            TH[32 * k + t, t, C:128] = g["theta"][k, 0, :]
    shared = {
        "posT": np.ascontiguousarray(g["pos_T"], np.float32),
        "gTr": g["gT"].reshape(1, N).astype(np.float32),
        "bTr": g["bT"].reshape(1, N).astype(np.float32),
        "wq": np.ascontiguousarray(g["wq"], np.float32),
        "wk": np.ascontiguousarray(g["wk"], np.float32),
        "wv": np.ascontiguousarray(g["wv"], np.float32),
        "fc": np.ascontiguousarray(g["fc"], np.float32),
        "prewT": np.ascontiguousarray(g["pre_w"][:, :, 0].T, np.float32),
        "posSb": (g["pos_S"] + g["pre_b"][None, :]).astype(np.float32),
        "gSr": g["gS"].reshape(1, DM).astype(np.float32),
        "bSr": g["bS"].reshape(1, DM).astype(np.float32),
        "swq": np.ascontiguousarray(g["swq"], np.float32),
        "swk": np.ascontiguousarray(g["swk"], np.float32),
        "Mh": (g["adj_pa"][None] * g["cmask"]).astype(np.float32),
        "cheb": np.ascontiguousarray(g["cheb"], np.float32),
        "TH": TH,
        "wp3": wp[3], "wp5": wp[5], "wp7": wp[7],
        "gbias": gbias,
        "fcw4": fcw4.astype(ml_dtypes.bfloat16),
        "rwRow": np.repeat(g["rw"][:, 0], T).reshape(1, C * T).astype(np.float32),
        "rbRow": np.repeat(g["rb"], T).reshape(1, C * T).astype(np.float32),
        "glnRow": np.repeat(g["gln"], T).reshape(1, C * T).astype(np.float32),
        "blnRow": np.repeat(g["bln"], T).reshape(1, C * T).astype(np.float32),
        "idm": np.eye(128, dtype=np.float32),
    }
    maps = []
    for c in range(NCORES):
        m = dict(shared)
        m["xb"] = x[c * BPC:(c + 1) * BPC]
        m["ra"] = ra[c * BPC:(c + 1) * BPC]
        m["raT"] = np.ascontiguousarray(np.transpose(ra[c * BPC:(c + 1) * BPC], (0, 1, 3, 2)))
        maps.append(m)
    return maps


def kernel(**inputs):
    global _PROG
    if _PROG is None:
        _PROG = build_program()
    maps = _host_prep(inputs)
    res = run_bass_kernel_spmd(_PROG, maps, list(range(NCORES)))
    outs = res.results
    out = np.concatenate([outs[c]["out"] for c in range(NCORES)], axis=0)
    sco = np.concatenate([outs[c]["sco"] for c in range(NCORES)], axis=0)
    return out.astype(np.float32), sco.reshape(B, 1, H, T, T).astype(np.float32)
